# revision 14
# baseline (speedup 1.0000x reference)
"""Trainium2 Bass kernel for MERITS_T (B=1024 data-parallel over 8 cores), v3.

v2 + precision/layout tuning driven by the v2 trace (PE 53us busy all-fp32,
AR starting at 70us of a 114us span):
  - bf16 host payloads for every large tensor (fp32 matmul streams at half
    rate on the PE and doubles DMA bytes; bf16 pipeline sims at 2.7e-3 rel
    err vs the 2e-2 gate).
  - all big host arrays pre-tiled to [128, t, n] so each DMA is one fat
    contiguous per-partition transfer (no sub-512B strided segments).
  - bf16 AllReduce (297KB -> 148KB payload).
  - patient-chain DMAs emitted before the w1 shard so the attention chain
    starts immediately; w1 reduce matmuls in bf16 overlap the shard DMA.
"""

import numpy as np
import ml_dtypes

import concourse.bass as bass
import concourse.mybir as mybir
from concourse.bass_utils import run_bass_kernel_spmd
from concourse.tile import TileContext

F32 = mybir.dt.float32
BF16 = mybir.dt.bfloat16
FP8 = mybir.dt.float8e4
AF = mybir.ActivationFunctionType
ALU = mybir.AluOpType
AX = mybir.AxisListType
BF = ml_dtypes.bfloat16
F8 = ml_dtypes.float8_e4m3fn


def split_multi_waits(nc):
    """The walrus on this image encodes at most ONE sync wait per TPB
    instruction. Hoist excess waits onto standalone InstEventSemaphore ops."""
    wid = 0
    for f in nc.m.functions:
        for bb in f.blocks:
            out = []
            for ins in bb.instructions:
                si = ins.sync_info
                if si is not None and si.on_wait and len(si.on_wait) > 1:
                    waits = list(si.on_wait)
                    for w in waits[:-1]:
                        wid += 1
                        out.append(mybir.InstEventSemaphore(
                            name=f"Wsplit-{wid}", engine=ins.engine,
                            ins=[], outs=[],
                            sync_info=mybir.SyncInfo(on_wait=[w], on_update=[])))
                    si.on_wait = waits[-1:]
                out.append(ins)
            bb.instructions = out
    return wid


B, T, MED, LAB, GLU, D, H = 1024, 25, 145, 1956, 16, 64, 32
NC_CORES = 8
BC = B // NC_CORES  # 128 patients per core
NH = 4
HID = MED * D // 8  # 1160
MBLK = 19  # ceil(145/8) out_w1 blocks per core (zero-padded)


def build_bass(split_waits=True):
    nc = bass.Bass()

    def inp(name, shape, dt=F32):
        return nc.dram_tensor(name, list(shape), dt, kind="ExternalInput")

    # ---- I/O (per-core shapes; layout + dtype marshalling done on host) ----
    w1s_d = inp("w1shard", (128, 8, 580), FP8)      # blocks 0-7, PE-summed
    w1s2_d = inp("w1shard2", (128, 11, 580), BF16)  # blocks 8-18, DVE-summed
    labT_d = inp("labT", (128, 16, 128), BF16)      # [k, t, p]
    sllw1_d = inp("sll_w1p", (128, 16, D), BF16)    # [k, t, d]
    sllw2_d = inp("sll_w2p", (D + 1, H))            # fp32, tiny
    gluT_d = inp("gluT", (128, 4, 128), BF16)       # [k, c, p]
    tfT_d = inp("tfT", (128, 4, 128), BF16)
    gluw_d = inp("glu_w16", (2 * GLU, H), BF16)
    glubt_d = inp("glu_bt", (1, T * H), BF16)       # glu_b tiled 25x
    glug_d = inp("glu_gate16", (1, H), BF16)
    med0T_d = inp("med0T", (146, 128))              # fp32 raw (exact is_gt)
    medw_d = inp("med_wp", (MED + 1, D), BF16)      # med_w + bias row
    medg_d = inp("med_gate", (1, D))                # fp32
    wq4_d = inp("wq4", (16, NH, D))                 # m1_wq^T as [d, h, e]
    wk4_d = inp("wk4", (16, NH, D))
    wv4_d = inp("wv4", (16, NH, D))
    wo4_d = inp("wo4", (16, NH, D))                 # m1_wo as [d, h, e']
    m2wvT_d = inp("m2wvT", (D, D))
    m2wo_d = inp("m2wo", (D, D))
    outb1_d = inp("out_b116", (1, HID), BF16)
    ow2_d = inp("ow2p", (128, 10, MED), BF16)       # [k, t, n]
    out_d = nc.dram_tensor("out", [BC, MED], F32, kind="ExternalOutput")

    # Inline constants
    bdm = np.zeros((128, 8 * H), np.float32)
    for jl in range(8):
        bdm[jl * GLU:(jl + 1) * GLU, jl * H:(jl + 1) * H] = 1.0
    bdmask_d = nc.inline_tensor(bdm.astype(BF), name="bd_mask")
    ident_d = nc.inline_tensor(np.eye(128, dtype=np.float32), name="ident128")
    identb_d = nc.inline_tensor(np.eye(128, dtype=F8), name="identb")

    # Internal DRAM for the W1 AllReduce ([(f h), i] layout, bf16)
    cc_in = nc.dram_tensor("cc_in", [128, 580], BF16)
    cc_out = nc.dram_tensor("cc_out", [128, 580], BF16, addr_space="Shared")

    with TileContext(nc) as tc, \
            tc.tile_pool(name="consts", bufs=1) as cp, \
            tc.tile_pool(name="ps", bufs=2, space="PSUM") as ps, \
            tc.tile_pool(name="psg", bufs=1, space="PSUM") as psg, \
            tc.tile_pool(name="psw", bufs=1, space="PSUM") as psw, \
            tc.tile_pool(name="pso", bufs=1, space="PSUM") as pso:

        dma = nc.sync.dma_start

        # Warm the ACT PWP tables for every function used later so the
        # 1.3-1.5us table loads happen at t=0, off the critical chains.
        warm = cp.tile([1, 4], F32, tag="warm")
        nc.vector.memset(warm, 0.0)
        for fn in (AF.Tanh, AF.Sigmoid, AF.Exp, AF.Relu, AF.Copy):
            nc.scalar.activation(out=warm, in_=warm, func=fn)

        # ============ w1 chain absolutely first (gates the AllReduce) =======
        # Split the 19-block sum: PE matmul-reduces 8 fp8 blocks while the
        # otherwise-idle DVE tree-sums 11 bf16 blocks in parallel; one final
        # DVE add merges. Cuts the collective-gating chain by ~6us.
        identb = cp.tile([128, 128], FP8, tag="identb")
        dma(out=identb, in_=identb_d[:])
        w1raw2 = cp.tile([128, 11, 580], BF16, tag="w1raw2")
        for (a, b) in [(0, 6), (6, 11)]:
            dma(out=w1raw2[:, a:b, :], in_=w1s2_d[:, a:b, :])
        w1raw = cp.tile([128, 8, 580], FP8, tag="w1raw")
        wred_ps = psw.tile([128, 580], F32, tag="w1red")
        for (a, b) in [(0, 4), (4, 8)]:
            dma(out=w1raw[:, a:b, :], in_=w1s_d[:, a:b, :])
            for m in range(a, b):
                nc.tensor.matmul(wred_ps[:, 0:512], lhsT=identb[:],
                                 rhs=w1raw[:, m, 0:512],
                                 start=(m == 0), stop=(m == 7))
                nc.tensor.matmul(wred_ps[:, 512:580], lhsT=identb[:],
                                 rhs=w1raw[:, m, 512:580],
                                 start=(m == 0), stop=(m == 7))

        def dvt(tag):
            t = cp.tile([128, 580], BF16, tag=tag, name=tag)
            return t

        with nc.allow_low_precision("w1 blocks arrive bf16; tree-sum in bf16"):
            badd = nc.vector.tensor_add
            b_ = lambda m: w1raw2[:, m, :]
            s0 = dvt("dv0")
            s1 = dvt("dv1")
            s2 = dvt("dv2")
            s3 = dvt("dv3")
            s4 = dvt("dv4")
            badd(s0, b_(0), b_(1))
            badd(s1, b_(2), b_(3))
            badd(s2, b_(4), b_(5))
            badd(s3, b_(6), b_(7))
            badd(s4, b_(8), b_(9))
            s5 = dvt("dv5")
            s6 = dvt("dv6")
            s7 = dvt("dv7")
            s8 = dvt("dv8")
            badd(s5, s0, s1)
            badd(s6, s2, s3)
            badd(s7, s4, b_(10))
            badd(s8, s5, s6)
            dsum = dvt("dsum")
            badd(dsum, s8, s7)
            w1red_a = cp.tile([128, 580], BF16, tag="w1red_a")
            nc.scalar.copy(out=w1red_a, in_=wred_ps)
            w1red = cp.tile([128, 580], BF16, tag="w1red_sb")
            badd(w1red, w1red_a, dsum)
        dma(out=cc_in[:], in_=w1red[:])
        nc.gpsimd.collective_compute(
            "AllReduce", ALU.add, replica_groups=[list(range(NC_CORES))],
            ins=[cc_in[:]], outs=[cc_out[:]])
        w1s_sb = cp.tile([D + 1, 1280], BF16, tag="w1s_sb")
        dma(out=w1s_sb[0:D, 0:HID],
            in_=cc_out[:].rearrange("(f h) i -> f (h i)", h=2))
        dma(out=w1s_sb[D:D + 1, 0:HID], in_=outb1_d[:])
        nc.vector.memset(w1s_sb[:, HID:1280], 0.0)
        # hid col 1160 must become the all-ones bias row of hidT: rrT row 64
        # is 1, so w1s_sb[64, 1160] = 1 makes relu(matmul) produce it.
        nc.vector.memset(w1s_sb[D:D + 1, HID:HID + 1], 1.0)

        # ============ patient-chain DMAs (small, unblock compute) ===========
        ident = cp.tile([128, 128], F32, tag="ident")
        dma(out=ident, in_=ident_d[:])
        gluT = cp.tile([128, 4, 128], BF16, tag="gluT")
        dma(out=gluT, in_=gluT_d[:])
        tfT = cp.tile([128, 4, 128], BF16, tag="tfT")
        dma(out=tfT, in_=tfT_d[:])
        m0a = cp.tile([128, 128], F32, tag="m0a")
        dma(out=m0a, in_=med0T_d[0:128, :])
        m0b = cp.tile([18, 128], F32, tag="m0b")
        dma(out=m0b, in_=med0T_d[128:146, :])
        bdmask = cp.tile([128, 8 * H], BF16, tag="bdmask")
        dma(out=bdmask, in_=bdmask_d[:])
        gw_g3 = cp.tile([GLU, H], BF16, tag="gw_g3")
        dma(out=gw_g3, in_=gluw_d[0:GLU, :])
        gw_t3 = cp.tile([GLU, H], BF16, tag="gw_t3")
        dma(out=gw_t3, in_=gluw_d[GLU:2 * GLU, :])
        glubt = cp.tile([1, T * H], BF16, tag="glubt")
        dma(out=glubt, in_=glubt_d[:])
        ggb = cp.tile([128, H], BF16, tag="ggb")
        dma(out=ggb, in_=glug_d[:].broadcast_to((128, H)))
        medw = cp.tile([128, D], BF16, tag="medw")
        dma(out=medw, in_=medw_d[0:128, :])
        medw2 = cp.tile([18, D], BF16, tag="medw2")
        dma(out=medw2, in_=medw_d[128:146, :])
        mgb = cp.tile([128, D], F32, tag="mgb")
        dma(out=mgb, in_=medg_d[:].broadcast_to((128, D)))
        wq4 = cp.tile([16, NH, D], F32, tag="wq4")
        dma(out=wq4, in_=wq4_d[:])
        wk4 = cp.tile([16, NH, D], F32, tag="wk4")
        dma(out=wk4, in_=wk4_d[:])
        wv4 = cp.tile([16, NH, D], F32, tag="wv4")
        dma(out=wv4, in_=wv4_d[:])
        wo4 = cp.tile([16, NH, D], F32, tag="wo4")
        dma(out=wo4, in_=wo4_d[:])
        m2wvT = cp.tile([D, D], F32, tag="m2wvT")
        dma(out=m2wvT, in_=m2wvT_d[:])
        m2wo = cp.tile([D, D], F32, tag="m2wo")
        dma(out=m2wo, in_=m2wo_d[:])
        w2sb = cp.tile([D + 1, H], F32, tag="w2sb")
        dma(out=w2sb, in_=sllw2_d[:])

        # static/tail-chain DMAs ride along after the shard
        labT = cp.tile([128, 16, 128], BF16, tag="labT")
        dma(out=labT, in_=labT_d[:])
        w1sb = cp.tile([128, 16, D], BF16, tag="w1sb")
        dma(out=w1sb, in_=sllw1_d[:])
        ow2sb = cp.tile([128, 10, MED], BF16, tag="ow2sb")
        dma(out=ow2sb, in_=ow2_d[:])

        # ============ attention weight prep (tiny fp32 PE matmuls) ==========
        ah_ps = ps.tile([D, NH, H], F32, tag="acc")
        for h in range(NH):
            nc.tensor.matmul(ah_ps[:, h, :], lhsT=wq4[:, h, :],
                             rhs=wk4[:, h, 0:H], start=True, stop=True)
        ah_sb = cp.tile([D, NH, H], F32, tag="ah_sb")
        nc.scalar.activation(out=ah_sb, in_=ah_ps, func=AF.Copy, scale=0.25)
        mh_ps = ps.tile([D, NH, D], F32, tag="acc")
        for h in range(NH):
            nc.tensor.matmul(mh_ps[:, h, :], lhsT=wo4[:, h, :],
                             rhs=wv4[:, h, :], start=True, stop=True)
        mhT = cp.tile([D, NH, D], F32, tag="mhT")
        nc.scalar.copy(out=mhT, in_=mh_ps)
        wvo_ps = ps.tile([D, D], F32, tag="acc")
        nc.tensor.matmul(wvo_ps, lhsT=m2wvT[:], rhs=m2wo[:],
                         start=True, stop=True)
        wvo = cp.tile([D, D], F32, tag="wvo")
        nc.scalar.copy(out=wvo, in_=wvo_ps)
        mws_ps = ps.tile([128, D], F32, tag="acc")
        for h in range(NH):
            nc.tensor.matmul(mws_ps[h * H:(h + 1) * H, :],
                             lhsT=mhT[:, h, 0:H], rhs=wvo[:],
                             start=True, stop=True, tile_position=(0, h * H))
        mw_stack = cp.tile([128, D], F32, tag="mw_stack")
        nc.scalar.copy(out=mw_stack, in_=mws_ps)
        msum_ps = ps.tile([H, D], F32, tag="acc")
        for h in range(NH):
            nc.tensor.matmul(msum_ps, lhsT=mhT[:, h, H:D], rhs=wvo[:],
                             start=(h == 0), stop=(h == NH - 1))
        msum = cp.tile([H, D], F32, tag="msum")
        nc.scalar.copy(out=msum, in_=msum_ps)

        # ============ glu encoder ===========================================
        def build_wbd(row0, tag):
            rep = cp.tile([128, H], BF16, tag=tag + "_rep")
            dma(out=rep,
                in_=gluw_d[row0:row0 + GLU, :].unsqueeze(0).broadcast_to((8, GLU, H)))
            wbd = cp.tile([128, 8, H], BF16, tag=tag)
            nc.vector.tensor_mul(wbd,
                                 rep[:].unsqueeze(1).broadcast_to((128, 8, H)),
                                 bdmask[:].rearrange("p (j o) -> p j o", j=8))
            return wbd

        wbd_g = build_wbd(0, "wbd_g")
        wbd_t = build_wbd(GLU, "wbd_t")
        ones1 = cp.tile([1, 128], BF16, tag="ones1")
        nc.vector.memset(ones1, 1.0)

        gx_ps = psg.tile([128, T, H], F32, tag="gx")
        for c in range(3):
            nc.tensor.matmul(gx_ps[:, 8 * c:8 * c + 8, :], lhsT=ones1[0:1, :],
                             rhs=glubt[0:1, 256 * c:256 * c + 256],
                             start=True, stop=False)
            nc.tensor.matmul(gx_ps[:, 8 * c:8 * c + 8, :], lhsT=gluT[:, c, :],
                             rhs=wbd_g[:], start=False, stop=False)
            nc.tensor.matmul(gx_ps[:, 8 * c:8 * c + 8, :], lhsT=tfT[:, c, :],
                             rhs=wbd_t[:], start=False, stop=True)
        nc.tensor.matmul(gx_ps[:, 24, :], lhsT=ones1[0:1, :],
                         rhs=glubt[0:1, 768:800], start=True, stop=False)
        nc.tensor.matmul(gx_ps[:, 24, :], lhsT=gluT[0:GLU, 3, :],
                         rhs=gw_g3[:], start=False, stop=False)
        nc.tensor.matmul(gx_ps[:, 24, :], lhsT=tfT[0:GLU, 3, :],
                         rhs=gw_t3[:], start=False, stop=True)
        xg = cp.tile([128, T, H], BF16, tag="xg")
        nc.scalar.activation(out=xg, in_=gx_ps, func=AF.Tanh)
        gm = cp.tile([128, T, H], BF16, tag="gm")
        nc.vector.tensor_mul(gm, xg,
                             ggb[:].unsqueeze(1).broadcast_to((128, T, H)))
        gs = cp.tile([128, T], F32, tag="gs")
        nc.vector.tensor_reduce(out=gs, in_=gm, axis=AX.X, op=ALU.add)
        gsg = cp.tile([128, T], F32, tag="gsg")
        nc.scalar.activation(out=gsg, in_=gs, func=AF.Sigmoid)

        # ============ med first-visit encoder -> mr0, u =====================
        mbTa = cp.tile([128, 128], BF16, tag="mbTa")
        nc.vector.tensor_scalar(out=mbTa, in0=m0a, scalar1=0.9, scalar2=None,
                                op0=ALU.is_gt)
        mbTb = cp.tile([18, 128], BF16, tag="mbTb")
        # host sets med0T row 145 to 1.0, so is_gt yields the bias ones row
        nc.vector.tensor_scalar(out=mbTb, in0=m0b, scalar1=0.9, scalar2=None,
                                op0=ALU.is_gt)
        x0_ps = ps.tile([128, D], F32, tag="acc")
        nc.tensor.matmul(x0_ps, lhsT=mbTa[:], rhs=medw[:],
                         start=True, stop=False)
        nc.tensor.matmul(x0_ps, lhsT=mbTb[:], rhs=medw2[:],
                         start=False, stop=True)
        x0 = cp.tile([128, D], F32, tag="x0")
        nc.vector.tensor_copy(out=x0, in_=x0_ps)
        scr = cp.tile([128, D], F32, tag="scr")
        nc.vector.tensor_mul(scr, x0, mgb)
        g0 = cp.tile([128, 1], F32, tag="g0")
        nc.vector.tensor_reduce(out=g0, in_=scr, axis=AX.X, op=ALU.add)
        sg0 = cp.tile([128, 1], F32, tag="sg0")
        nc.scalar.activation(out=sg0, in_=g0, func=AF.Sigmoid)
        mr0 = cp.tile([128, D], F32, tag="mr0")
        nc.vector.tensor_scalar(out=mr0, in0=x0, scalar1=sg0[:, 0:1],
                                scalar2=None, op0=ALU.mult)
        mr0T_ps = ps.tile([D, 128], F32, tag="acc")
        nc.tensor.transpose(mr0T_ps[0:D, 0:128], mr0[:], ident[:])
        mr0T = cp.tile([D, 128], F32, tag="mr0T")
        nc.vector.tensor_copy(out=mr0T, in_=mr0T_ps)
        u_ps = ps.tile([128, NH, H], F32, tag="acc")
        for h in range(NH):
            nc.tensor.matmul(u_ps[:, h, :], lhsT=mr0T[:], rhs=ah_sb[:, h, :],
                             start=True, stop=True)
        u16 = cp.tile([128, NH, H], BF16, tag="u16")
        nc.scalar.copy(out=u16, in_=u_ps)

        # ============ static MLP over lab (transposed throughout) ===========
        st1_ps = ps.tile([D, 128], F32, tag="acc")
        for kt in range(16):
            nc.tensor.matmul(st1_ps, lhsT=w1sb[:, kt, :], rhs=labT[:, kt, :],
                             start=(kt == 0), stop=(kt == 15))
        st1rT = cp.tile([D + 1, 128], F32, tag="st1rT")
        nc.scalar.activation(out=st1rT[0:D, :], in_=st1_ps, func=AF.Relu)
        nc.vector.memset(st1rT[D:D + 1, :], 1.0)
        st2_ps = ps.tile([H, 128], F32, tag="acc")
        nc.tensor.matmul(st2_ps, lhsT=w2sb[:], rhs=st1rT[:],
                         start=True, stop=True)
        stT = cp.tile([H, 128], F32, tag="stT")
        nc.scalar.activation(out=stT, in_=st2_ps, func=AF.Relu)

        # ============ one-query attention (glu half only) ===================
        sprod = cp.tile([128, T, NH, H], BF16, tag="sprod")
        nc.vector.tensor_mul(
            sprod,
            xg[:].unsqueeze(2).broadcast_to((128, T, NH, H)),
            u16[:].unsqueeze(1).broadcast_to((128, T, NH, H)))
        s_sb = cp.tile([128, T, NH], F32, tag="s_sb")
        nc.vector.tensor_reduce(out=s_sb, in_=sprod, axis=AX.X, op=ALU.add)
        sgat = cp.tile([128, T, NH], F32, tag="sgat")
        nc.vector.tensor_mul(sgat, s_sb,
                             gsg[:].unsqueeze(2).broadcast_to((128, T, NH)))
        es = cp.tile([128, T, NH], F32, tag="es")
        nc.scalar.activation(out=es, in_=sgat, func=AF.Exp)
        den = cp.tile([128, NH], F32, tag="den")
        nc.vector.tensor_reduce(out=den, in_=es.rearrange("p j h -> p h j"),
                                axis=AX.X, op=ALU.add)
        rden = cp.tile([128, NH], F32, tag="rden")
        nc.vector.reciprocal(out=rden, in_=den)
        attn1 = cp.tile([128, NH, T], F32, tag="attn1")
        nc.vector.tensor_mul(attn1, es.rearrange("p j h -> p h j"),
                             rden[:].unsqueeze(2).broadcast_to((128, NH, T)))
        attn2 = cp.tile([128, NH, T], BF16, tag="attn2")
        nc.vector.tensor_mul(attn2, attn1,
                             gsg[:].unsqueeze(1).broadcast_to((128, NH, T)))
        wprod = cp.tile([128, NH, T, H], BF16, tag="wprod")
        nc.vector.tensor_mul(
            wprod,
            xg[:].unsqueeze(1).broadcast_to((128, NH, T, H)),
            attn2[:].unsqueeze(3).broadcast_to((128, NH, T, H)))
        w_sb = cp.tile([128, NH, H], F32, tag="w_sb")
        nc.vector.tensor_reduce(out=w_sb,
                                in_=wprod.rearrange("p h j f -> p h f j"),
                                axis=AX.X, op=ALU.add)
        wT_ps = ps.tile([128, 128], F32, tag="acc")
        nc.tensor.transpose(wT_ps, w_sb[:].rearrange("p h f -> p (h f)"),
                            ident[:])
        wT = cp.tile([128, 128], F32, tag="wT")
        nc.vector.tensor_copy(out=wT, in_=wT_ps)
        rT_ps = ps.tile([D, 128], F32, tag="acc")
        nc.tensor.matmul(rT_ps, lhsT=mw_stack[:], rhs=wT[:],
                         start=True, stop=False)
        nc.tensor.matmul(rT_ps, lhsT=msum[:], rhs=stT[:],
                         start=False, stop=True)
        rrT = cp.tile([D + 1, 128], BF16, tag="rrT")
        nc.scalar.activation(out=rrT[0:D, :], in_=rT_ps, func=AF.Relu)
        nc.vector.memset(rrT[D:D + 1, :], 1.0)

        # ============ final MLP (transposed hid, bf16) ======================
        # pairs of 128-col blocks per PSUM bank: 5 relus instead of 10, and
        # the accumulating out matmuls chase each pair as it lands.
        hidT = cp.tile([128, 10, 128], BF16, tag="hidT")
        out_ps = pso.tile([128, MED], F32, tag="out")
        for pb in range(5):
            h_ps = ps.tile([128, 2, 128], F32, tag="acc")
            for half in range(2):
                ob = 2 * pb + half
                nc.tensor.matmul(h_ps[:, half, :],
                                 lhsT=w1s_sb[:, ob * 128:(ob + 1) * 128],
                                 rhs=rrT[:], start=True, stop=True)
            nc.scalar.activation(out=hidT[:, 2 * pb:2 * pb + 2, :], in_=h_ps,
                                 func=AF.Relu)
            for half in range(2):
                ob = 2 * pb + half
                nc.tensor.matmul(out_ps, lhsT=hidT[:, ob, :],
                                 rhs=ow2sb[:, ob, :],
                                 start=(ob == 0), stop=(ob == 9))
        out_sb = cp.tile([128, MED], F32, tag="out_sb")
        nc.vector.tensor_copy(out=out_sb, in_=out_ps)
        dma(out=out_d[:], in_=out_sb)

    if split_waits:
        split_multi_waits(nc)
    return nc


_CACHED_NC = None


def make_in_maps(inputs):
    f = lambda x: np.ascontiguousarray(np.asarray(x, dtype=np.float32))
    bf = lambda x: np.ascontiguousarray(np.asarray(x, dtype=np.float32).astype(BF))
    c_ = np.concatenate

    # out_w1 -> per-core [(f h), m, i] shards, blocks 0-7 fp8 (PE) and
    # blocks 8-18 bf16 (DVE tree), zero-padded m
    w1b = f(inputs["out_w1"]).reshape(MED, D, 2, 580)
    w1pad = np.zeros((NC_CORES * MBLK, D, 2, 580), np.float32)
    w1pad[:MED] = w1b
    allb = (w1pad.reshape(NC_CORES, MBLK, D, 2, 580)
            .transpose(0, 2, 3, 1, 4).reshape(NC_CORES, 128, MBLK, 580))
    shards = np.ascontiguousarray(allb[:, :, 0:8, :].astype(F8))
    shards2 = np.ascontiguousarray(allb[:, :, 8:19, :].astype(BF))

    def tile3(arr2d, n_t, width):
        """[n_t*128, width] row-major -> [128, n_t, width] (k, t, w)."""
        return np.ascontiguousarray(
            arr2d.reshape(n_t, 128, width).transpose(1, 0, 2))

    # lab^T with ones row (bias fold), zero-padded to 2048 rows, tiled
    lab = f(inputs["lab"])
    labT = np.zeros((2048, B), np.float32)
    labT[:LAB] = lab.T
    labT[LAB] = 1.0

    glu = f(inputs["glu"]).reshape(B, T * GLU)
    tf = f(inputs["time_feat"]).reshape(B, T * GLU)
    gluT = np.zeros((512, B), np.float32)
    gluT[:T * GLU] = glu.T
    tfT = np.zeros((512, B), np.float32)
    tfT[:T * GLU] = tf.T

    med0T = np.zeros((146, B), np.float32)
    med0T[:MED] = f(inputs["med"])[:, 0, :].T
    med0T[MED] = 1.0  # > 0.9 threshold -> bias ones row after is_gt

    sll_w1p = np.zeros((2048, D), np.float32)
    sll_w1p[:LAB] = f(inputs["sll_w1"])
    sll_w1p[LAB] = f(inputs["sll_b1"])

    ow2p = np.zeros((1280, MED), np.float32)
    ow2p[:HID] = f(inputs["out_w2"])
    ow2p[HID] = f(inputs["out_b2"])

    head = lambda w: np.ascontiguousarray(
        w.reshape(NH, 16, D).transpose(1, 0, 2))

    rep = {
        "sll_w1p": tile3(sll_w1p, 16, D).astype(BF),
        "sll_w2p": c_([f(inputs["sll_w2"]), f(inputs["sll_b2"]).reshape(1, H)], 0),
        "glu_w16": bf(inputs["glu_w"]),
        "glu_bt": np.ascontiguousarray(
            np.tile(f(inputs["glu_b"]).reshape(1, H), (1, T)).astype(BF)),
        "glu_gate16": bf(inputs["glu_gate"]).reshape(1, H),
        "med_wp": c_([f(inputs["med_w"]), f(inputs["med_b"]).reshape(1, D)],
                     0).astype(BF),
        "med_gate": f(inputs["med_gate"]).reshape(1, D),
        "wq4": head(f(inputs["m1_wq"]).T),
        "wk4": head(f(inputs["m1_wk"]).T),
        "wv4": head(f(inputs["m1_wv"]).T),
        "wo4": head(f(inputs["m1_wo"])),
        "m2wvT": np.ascontiguousarray(f(inputs["m2_wv"]).T),
        "m2wo": f(inputs["m2_wo"]),
        "out_b116": bf(inputs["out_b1"]).reshape(1, HID),
        "ow2p": tile3(ow2p, 10, MED).astype(BF),
    }

    in_maps = []
    for c in range(NC_CORES):
        sl = slice(c * BC, (c + 1) * BC)
        in_maps.append({
            "w1shard": shards[c],
            "w1shard2": shards2[c],
            "labT": tile3(np.ascontiguousarray(labT[:, sl]), 16, 128).astype(BF),
            "gluT": tile3(np.ascontiguousarray(gluT[:, sl]), 4, 128).astype(BF),
            "tfT": tile3(np.ascontiguousarray(tfT[:, sl]), 4, 128).astype(BF),
            "med0T": np.ascontiguousarray(med0T[:, sl]),
            **rep,
        })
    return in_maps


def kernel(**inputs):
    global _CACHED_NC
    if _CACHED_NC is None:
        _CACHED_NC = build_bass()
    nc = _CACHED_NC
    in_maps = make_in_maps(inputs)
    res = run_bass_kernel_spmd(nc, in_maps, core_ids=list(range(NC_CORES)))
    return np.concatenate([res.results[c]["out"] for c in range(NC_CORES)], axis=0)


if __name__ == "__main__":
    import reference
    inp = reference.setup_inputs()
    out = kernel(**{k: np.asarray(v) for k, v in inp.items()})
    print("kernel output", out.shape, out.dtype)


# revision 15
# speedup vs baseline: 1.0700x; 1.0700x over previous
"""Trainium2 Bass kernel for MERITS_T (B=1024 data-parallel over 8 cores), v3.

v2 + precision/layout tuning driven by the v2 trace (PE 53us busy all-fp32,
AR starting at 70us of a 114us span):
  - bf16 host payloads for every large tensor (fp32 matmul streams at half
    rate on the PE and doubles DMA bytes; bf16 pipeline sims at 2.7e-3 rel
    err vs the 2e-2 gate).
  - all big host arrays pre-tiled to [128, t, n] so each DMA is one fat
    contiguous per-partition transfer (no sub-512B strided segments).
  - bf16 AllReduce (297KB -> 148KB payload).
  - patient-chain DMAs emitted before the w1 shard so the attention chain
    starts immediately; w1 reduce matmuls in bf16 overlap the shard DMA.
"""

import numpy as np
import ml_dtypes

import concourse.bass as bass
import concourse.mybir as mybir
from concourse.bass_utils import run_bass_kernel_spmd
from concourse.tile import TileContext

F32 = mybir.dt.float32
BF16 = mybir.dt.bfloat16
FP8 = mybir.dt.float8e4
AF = mybir.ActivationFunctionType
ALU = mybir.AluOpType
AX = mybir.AxisListType
BF = ml_dtypes.bfloat16
F8 = ml_dtypes.float8_e4m3fn


def split_multi_waits(nc):
    """The walrus on this image encodes at most ONE sync wait per TPB
    instruction. Hoist excess waits onto standalone InstEventSemaphore ops."""
    wid = 0
    for f in nc.m.functions:
        for bb in f.blocks:
            out = []
            for ins in bb.instructions:
                si = ins.sync_info
                if si is not None and si.on_wait and len(si.on_wait) > 1:
                    waits = list(si.on_wait)
                    for w in waits[:-1]:
                        wid += 1
                        out.append(mybir.InstEventSemaphore(
                            name=f"Wsplit-{wid}", engine=ins.engine,
                            ins=[], outs=[],
                            sync_info=mybir.SyncInfo(on_wait=[w], on_update=[])))
                    si.on_wait = waits[-1:]
                out.append(ins)
            bb.instructions = out
    return wid


B, T, MED, LAB, GLU, D, H = 1024, 25, 145, 1956, 16, 64, 32
NC_CORES = 8
BC = B // NC_CORES  # 128 patients per core
NH = 4
HID = MED * D // 8  # 1160
MBLK = 19  # ceil(145/8) out_w1 blocks per core (zero-padded)


def build_bass(split_waits=True):
    nc = bass.Bass()

    def inp(name, shape, dt=F32):
        return nc.dram_tensor(name, list(shape), dt, kind="ExternalInput")

    # ---- I/O (per-core shapes; layout + dtype marshalling done on host) ----
    w1s_d = inp("w1shard", (128, MBLK, 580), FP8)   # [(f h), m, i], e4m3
    labT_d = inp("labT", (128, 16, 128), BF16)      # [k, t, p]
    sllw1_d = inp("sll_w1p", (128, 16, D), BF16)    # [k, t, d]
    sllw2_d = inp("sll_w2p", (D + 1, H))            # fp32, tiny
    gluT_d = inp("gluT", (128, 4, 128), BF16)       # [k, c, p]
    tfT_d = inp("tfT", (128, 4, 128), BF16)
    gluw_d = inp("glu_w16", (2 * GLU, H), BF16)
    glubt_d = inp("glu_bt", (1, T * H), BF16)       # glu_b tiled 25x
    glug_d = inp("glu_gate16", (1, H), BF16)
    med0T_d = inp("med0T", (146, 128))              # fp32 raw (exact is_gt)
    medw_d = inp("med_wp", (MED + 1, D), BF16)      # med_w + bias row
    medg_d = inp("med_gate", (1, D))                # fp32
    wq4_d = inp("wq4", (16, NH, D))                 # m1_wq^T as [d, h, e]
    wk4_d = inp("wk4", (16, NH, D))
    wv4_d = inp("wv4", (16, NH, D))
    wo4_d = inp("wo4", (16, NH, D))                 # m1_wo as [d, h, e']
    m2wvT_d = inp("m2wvT", (D, D))
    m2wo_d = inp("m2wo", (D, D))
    outb1_d = inp("out_b116", (1, HID), BF16)
    ow2_d = inp("ow2p", (128, 10, MED), BF16)       # [k, t, n]
    out_d = nc.dram_tensor("out", [BC, MED], F32, kind="ExternalOutput")

    # Inline constants
    bdm = np.zeros((128, 8 * H), np.float32)
    for jl in range(8):
        bdm[jl * GLU:(jl + 1) * GLU, jl * H:(jl + 1) * H] = 1.0
    bdmask_d = nc.inline_tensor(bdm.astype(BF), name="bd_mask")
    ident_d = nc.inline_tensor(np.eye(128, dtype=np.float32), name="ident128")
    identb_d = nc.inline_tensor(np.eye(128, dtype=F8), name="identb")

    # Internal DRAM for the W1 AllReduce ([(f h), i] layout, bf16)
    cc_in = nc.dram_tensor("cc_in", [128, 580], BF16)
    cc_out = nc.dram_tensor("cc_out", [128, 580], BF16, addr_space="Shared")

    with TileContext(nc) as tc, \
            tc.tile_pool(name="consts", bufs=1) as cp, \
            tc.tile_pool(name="ps", bufs=2, space="PSUM") as ps, \
            tc.tile_pool(name="psg", bufs=1, space="PSUM") as psg, \
            tc.tile_pool(name="psw", bufs=1, space="PSUM") as psw, \
            tc.tile_pool(name="pso", bufs=1, space="PSUM") as pso:

        dma = nc.sync.dma_start

        # Warm the ACT PWP tables for every function used later so the
        # 1.3-1.5us table loads happen at t=0, off the critical chains.
        warm = cp.tile([1, 4], F32, tag="warm")
        nc.vector.memset(warm, 0.0)
        for fn in (AF.Tanh, AF.Sigmoid, AF.Exp, AF.Relu, AF.Copy):
            nc.scalar.activation(out=warm, in_=warm, func=fn)

        # ============ w1 chain absolutely first (gates the AllReduce) =======
        identb = cp.tile([128, 128], FP8, tag="identb")
        dma(out=identb, in_=identb_d[:])
        w1raw = cp.tile([128, MBLK, 580], FP8, tag="w1raw")
        wred_ps = psw.tile([128, 580], F32, tag="w1red")
        chunks = [(0, 4), (4, 8), (8, 12), (12, 17), (17, 19)]
        for (a, b) in chunks:
            dma(out=w1raw[:, a:b, :], in_=w1s_d[:, a:b, :])
            for m in range(a, b):
                nc.tensor.matmul(wred_ps[:, 0:512], lhsT=identb[:],
                                 rhs=w1raw[:, m, 0:512],
                                 start=(m == 0), stop=(m == MBLK - 1))
                nc.tensor.matmul(wred_ps[:, 512:580], lhsT=identb[:],
                                 rhs=w1raw[:, m, 512:580],
                                 start=(m == 0), stop=(m == MBLK - 1))
        w1red = cp.tile([128, 580], BF16, tag="w1red_sb")
        nc.scalar.copy(out=w1red, in_=wred_ps)
        dma(out=cc_in[:], in_=w1red[:])
        nc.gpsimd.collective_compute(
            "AllReduce", ALU.add, replica_groups=[list(range(NC_CORES))],
            ins=[cc_in[:]], outs=[cc_out[:]])
        w1s_sb = cp.tile([D + 1, 1280], BF16, tag="w1s_sb")
        dma(out=w1s_sb[0:D, 0:HID],
            in_=cc_out[:].rearrange("(f h) i -> f (h i)", h=2))
        dma(out=w1s_sb[D:D + 1, 0:HID], in_=outb1_d[:])
        nc.vector.memset(w1s_sb[:, HID:1280], 0.0)
        # hid col 1160 must become the all-ones bias row of hidT: rrT row 64
        # is 1, so w1s_sb[64, 1160] = 1 makes relu(matmul) produce it.
        nc.vector.memset(w1s_sb[D:D + 1, HID:HID + 1], 1.0)

        # ============ patient-chain DMAs (small, unblock compute) ===========
        ident = cp.tile([128, 128], F32, tag="ident")
        dma(out=ident, in_=ident_d[:])
        gluT = cp.tile([128, 4, 128], BF16, tag="gluT")
        dma(out=gluT, in_=gluT_d[:])
        tfT = cp.tile([128, 4, 128], BF16, tag="tfT")
        dma(out=tfT, in_=tfT_d[:])
        m0a = cp.tile([128, 128], F32, tag="m0a")
        dma(out=m0a, in_=med0T_d[0:128, :])
        m0b = cp.tile([18, 128], F32, tag="m0b")
        dma(out=m0b, in_=med0T_d[128:146, :])
        bdmask = cp.tile([128, 8 * H], BF16, tag="bdmask")
        dma(out=bdmask, in_=bdmask_d[:])
        gw_g3 = cp.tile([GLU, H], BF16, tag="gw_g3")
        dma(out=gw_g3, in_=gluw_d[0:GLU, :])
        gw_t3 = cp.tile([GLU, H], BF16, tag="gw_t3")
        dma(out=gw_t3, in_=gluw_d[GLU:2 * GLU, :])
        glubt = cp.tile([1, T * H], BF16, tag="glubt")
        dma(out=glubt, in_=glubt_d[:])
        ggb = cp.tile([128, H], BF16, tag="ggb")
        dma(out=ggb, in_=glug_d[:].broadcast_to((128, H)))
        medw = cp.tile([128, D], BF16, tag="medw")
        dma(out=medw, in_=medw_d[0:128, :])
        medw2 = cp.tile([18, D], BF16, tag="medw2")
        dma(out=medw2, in_=medw_d[128:146, :])
        mgb = cp.tile([128, D], F32, tag="mgb")
        dma(out=mgb, in_=medg_d[:].broadcast_to((128, D)))
        wq4 = cp.tile([16, NH, D], F32, tag="wq4")
        dma(out=wq4, in_=wq4_d[:])
        wk4 = cp.tile([16, NH, D], F32, tag="wk4")
        dma(out=wk4, in_=wk4_d[:])
        wv4 = cp.tile([16, NH, D], F32, tag="wv4")
        dma(out=wv4, in_=wv4_d[:])
        wo4 = cp.tile([16, NH, D], F32, tag="wo4")
        dma(out=wo4, in_=wo4_d[:])
        m2wvT = cp.tile([D, D], F32, tag="m2wvT")
        dma(out=m2wvT, in_=m2wvT_d[:])
        m2wo = cp.tile([D, D], F32, tag="m2wo")
        dma(out=m2wo, in_=m2wo_d[:])
        w2sb = cp.tile([D + 1, H], F32, tag="w2sb")
        dma(out=w2sb, in_=sllw2_d[:])

        # static/tail-chain DMAs ride along after the shard
        labT = cp.tile([128, 16, 128], BF16, tag="labT")
        dma(out=labT, in_=labT_d[:])
        w1sb = cp.tile([128, 16, D], BF16, tag="w1sb")
        dma(out=w1sb, in_=sllw1_d[:])
        ow2sb = cp.tile([128, 10, MED], BF16, tag="ow2sb")
        dma(out=ow2sb, in_=ow2_d[:])

        # ============ attention weight prep (tiny fp32 PE matmuls) ==========
        ah_ps = ps.tile([D, NH, H], F32, tag="acc")
        for h in range(NH):
            nc.tensor.matmul(ah_ps[:, h, :], lhsT=wq4[:, h, :],
                             rhs=wk4[:, h, 0:H], start=True, stop=True)
        ah_sb = cp.tile([D, NH, H], F32, tag="ah_sb")
        nc.scalar.activation(out=ah_sb, in_=ah_ps, func=AF.Copy, scale=0.25)
        mh_ps = ps.tile([D, NH, D], F32, tag="acc")
        for h in range(NH):
            nc.tensor.matmul(mh_ps[:, h, :], lhsT=wo4[:, h, :],
                             rhs=wv4[:, h, :], start=True, stop=True)
        mhT = cp.tile([D, NH, D], F32, tag="mhT")
        nc.scalar.copy(out=mhT, in_=mh_ps)
        wvo_ps = ps.tile([D, D], F32, tag="acc")
        nc.tensor.matmul(wvo_ps, lhsT=m2wvT[:], rhs=m2wo[:],
                         start=True, stop=True)
        wvo = cp.tile([D, D], F32, tag="wvo")
        nc.scalar.copy(out=wvo, in_=wvo_ps)
        mws_ps = ps.tile([128, D], F32, tag="acc")
        for h in range(NH):
            nc.tensor.matmul(mws_ps[h * H:(h + 1) * H, :],
                             lhsT=mhT[:, h, 0:H], rhs=wvo[:],
                             start=True, stop=True, tile_position=(0, h * H))
        mw_stack = cp.tile([128, D], F32, tag="mw_stack")
        nc.scalar.copy(out=mw_stack, in_=mws_ps)
        msum_ps = ps.tile([H, D], F32, tag="acc")
        for h in range(NH):
            nc.tensor.matmul(msum_ps, lhsT=mhT[:, h, H:D], rhs=wvo[:],
                             start=(h == 0), stop=(h == NH - 1))
        msum = cp.tile([H, D], F32, tag="msum")
        nc.scalar.copy(out=msum, in_=msum_ps)

        # ============ glu encoder ===========================================
        def build_wbd(row0, tag):
            rep = cp.tile([128, H], BF16, tag=tag + "_rep")
            dma(out=rep,
                in_=gluw_d[row0:row0 + GLU, :].unsqueeze(0).broadcast_to((8, GLU, H)))
            wbd = cp.tile([128, 8, H], BF16, tag=tag)
            nc.vector.tensor_mul(wbd,
                                 rep[:].unsqueeze(1).broadcast_to((128, 8, H)),
                                 bdmask[:].rearrange("p (j o) -> p j o", j=8))
            return wbd

        wbd_g = build_wbd(0, "wbd_g")
        wbd_t = build_wbd(GLU, "wbd_t")
        ones1 = cp.tile([1, 128], BF16, tag="ones1")
        nc.vector.memset(ones1, 1.0)

        gx_ps = psg.tile([128, T, H], F32, tag="gx")
        for c in range(3):
            nc.tensor.matmul(gx_ps[:, 8 * c:8 * c + 8, :], lhsT=ones1[0:1, :],
                             rhs=glubt[0:1, 256 * c:256 * c + 256],
                             start=True, stop=False)
            nc.tensor.matmul(gx_ps[:, 8 * c:8 * c + 8, :], lhsT=gluT[:, c, :],
                             rhs=wbd_g[:], start=False, stop=False)
            nc.tensor.matmul(gx_ps[:, 8 * c:8 * c + 8, :], lhsT=tfT[:, c, :],
                             rhs=wbd_t[:], start=False, stop=True)
        nc.tensor.matmul(gx_ps[:, 24, :], lhsT=ones1[0:1, :],
                         rhs=glubt[0:1, 768:800], start=True, stop=False)
        nc.tensor.matmul(gx_ps[:, 24, :], lhsT=gluT[0:GLU, 3, :],
                         rhs=gw_g3[:], start=False, stop=False)
        nc.tensor.matmul(gx_ps[:, 24, :], lhsT=tfT[0:GLU, 3, :],
                         rhs=gw_t3[:], start=False, stop=True)
        xg = cp.tile([128, T, H], BF16, tag="xg")
        nc.scalar.activation(out=xg, in_=gx_ps, func=AF.Tanh)
        gm = cp.tile([128, T, H], BF16, tag="gm")
        nc.vector.tensor_mul(gm, xg,
                             ggb[:].unsqueeze(1).broadcast_to((128, T, H)))
        gs = cp.tile([128, T], F32, tag="gs")
        nc.vector.tensor_reduce(out=gs, in_=gm, axis=AX.X, op=ALU.add)
        gsg = cp.tile([128, T], F32, tag="gsg")
        nc.scalar.activation(out=gsg, in_=gs, func=AF.Sigmoid)

        # ============ med first-visit encoder -> mr0, u =====================
        mbTa = cp.tile([128, 128], BF16, tag="mbTa")
        nc.vector.tensor_scalar(out=mbTa, in0=m0a, scalar1=0.9, scalar2=None,
                                op0=ALU.is_gt)
        mbTb = cp.tile([18, 128], BF16, tag="mbTb")
        # host sets med0T row 145 to 1.0, so is_gt yields the bias ones row
        nc.vector.tensor_scalar(out=mbTb, in0=m0b, scalar1=0.9, scalar2=None,
                                op0=ALU.is_gt)
        x0_ps = ps.tile([128, D], F32, tag="acc")
        nc.tensor.matmul(x0_ps, lhsT=mbTa[:], rhs=medw[:],
                         start=True, stop=False)
        nc.tensor.matmul(x0_ps, lhsT=mbTb[:], rhs=medw2[:],
                         start=False, stop=True)
        x0 = cp.tile([128, D], F32, tag="x0")
        nc.vector.tensor_copy(out=x0, in_=x0_ps)
        scr = cp.tile([128, D], F32, tag="scr")
        nc.vector.tensor_mul(scr, x0, mgb)
        g0 = cp.tile([128, 1], F32, tag="g0")
        nc.vector.tensor_reduce(out=g0, in_=scr, axis=AX.X, op=ALU.add)
        sg0 = cp.tile([128, 1], F32, tag="sg0")
        nc.scalar.activation(out=sg0, in_=g0, func=AF.Sigmoid)
        mr0 = cp.tile([128, D], F32, tag="mr0")
        nc.vector.tensor_scalar(out=mr0, in0=x0, scalar1=sg0[:, 0:1],
                                scalar2=None, op0=ALU.mult)
        mr0T_ps = ps.tile([D, 128], F32, tag="acc")
        nc.tensor.transpose(mr0T_ps[0:D, 0:128], mr0[:], ident[:])
        mr0T = cp.tile([D, 128], F32, tag="mr0T")
        nc.vector.tensor_copy(out=mr0T, in_=mr0T_ps)
        u_ps = ps.tile([128, NH, H], F32, tag="acc")
        for h in range(NH):
            nc.tensor.matmul(u_ps[:, h, :], lhsT=mr0T[:], rhs=ah_sb[:, h, :],
                             start=True, stop=True)
        u16 = cp.tile([128, NH, H], BF16, tag="u16")
        nc.scalar.copy(out=u16, in_=u_ps)

        # ============ static MLP over lab (transposed throughout) ===========
        st1_ps = ps.tile([D, 128], F32, tag="acc")
        for kt in range(16):
            nc.tensor.matmul(st1_ps, lhsT=w1sb[:, kt, :], rhs=labT[:, kt, :],
                             start=(kt == 0), stop=(kt == 15))
        st1rT = cp.tile([D + 1, 128], F32, tag="st1rT")
        nc.scalar.activation(out=st1rT[0:D, :], in_=st1_ps, func=AF.Relu)
        nc.vector.memset(st1rT[D:D + 1, :], 1.0)
        st2_ps = ps.tile([H, 128], F32, tag="acc")
        nc.tensor.matmul(st2_ps, lhsT=w2sb[:], rhs=st1rT[:],
                         start=True, stop=True)
        stT = cp.tile([H, 128], F32, tag="stT")
        nc.scalar.activation(out=stT, in_=st2_ps, func=AF.Relu)

        # ============ one-query attention (glu half only) ===================
        sprod = cp.tile([128, T, NH, H], BF16, tag="sprod")
        nc.vector.tensor_mul(
            sprod,
            xg[:].unsqueeze(2).broadcast_to((128, T, NH, H)),
            u16[:].unsqueeze(1).broadcast_to((128, T, NH, H)))
        s_sb = cp.tile([128, T, NH], F32, tag="s_sb")
        nc.vector.tensor_reduce(out=s_sb, in_=sprod, axis=AX.X, op=ALU.add)
        sgat = cp.tile([128, T, NH], F32, tag="sgat")
        nc.vector.tensor_mul(sgat, s_sb,
                             gsg[:].unsqueeze(2).broadcast_to((128, T, NH)))
        es = cp.tile([128, T, NH], F32, tag="es")
        nc.scalar.activation(out=es, in_=sgat, func=AF.Exp)
        den = cp.tile([128, NH], F32, tag="den")
        nc.vector.tensor_reduce(out=den, in_=es.rearrange("p j h -> p h j"),
                                axis=AX.X, op=ALU.add)
        rden = cp.tile([128, NH], F32, tag="rden")
        nc.vector.reciprocal(out=rden, in_=den)
        attn1 = cp.tile([128, NH, T], F32, tag="attn1")
        nc.vector.tensor_mul(attn1, es.rearrange("p j h -> p h j"),
                             rden[:].unsqueeze(2).broadcast_to((128, NH, T)))
        attn2 = cp.tile([128, NH, T], BF16, tag="attn2")
        nc.vector.tensor_mul(attn2, attn1,
                             gsg[:].unsqueeze(1).broadcast_to((128, NH, T)))
        wprod = cp.tile([128, NH, T, H], BF16, tag="wprod")
        nc.vector.tensor_mul(
            wprod,
            xg[:].unsqueeze(1).broadcast_to((128, NH, T, H)),
            attn2[:].unsqueeze(3).broadcast_to((128, NH, T, H)))
        w_sb = cp.tile([128, NH, H], F32, tag="w_sb")
        nc.vector.tensor_reduce(out=w_sb,
                                in_=wprod.rearrange("p h j f -> p h f j"),
                                axis=AX.X, op=ALU.add)
        wT_ps = ps.tile([128, 128], F32, tag="acc")
        nc.tensor.transpose(wT_ps, w_sb[:].rearrange("p h f -> p (h f)"),
                            ident[:])
        wT = cp.tile([128, 128], F32, tag="wT")
        nc.vector.tensor_copy(out=wT, in_=wT_ps)
        rT_ps = ps.tile([D, 128], F32, tag="acc")
        nc.tensor.matmul(rT_ps, lhsT=mw_stack[:], rhs=wT[:],
                         start=True, stop=False)
        nc.tensor.matmul(rT_ps, lhsT=msum[:], rhs=stT[:],
                         start=False, stop=True)
        rrT = cp.tile([D + 1, 128], BF16, tag="rrT")
        nc.scalar.activation(out=rrT[0:D, :], in_=rT_ps, func=AF.Relu)
        nc.vector.memset(rrT[D:D + 1, :], 1.0)

        # ============ final MLP (transposed hid, bf16) ======================
        # pairs of 128-col blocks per PSUM bank: 5 relus instead of 10, and
        # the accumulating out matmuls chase each pair as it lands.
        hidT = cp.tile([128, 10, 128], BF16, tag="hidT")
        out_ps = pso.tile([128, MED], F32, tag="out")
        for pb in range(5):
            h_ps = ps.tile([128, 2, 128], F32, tag="acc")
            for half in range(2):
                ob = 2 * pb + half
                nc.tensor.matmul(h_ps[:, half, :],
                                 lhsT=w1s_sb[:, ob * 128:(ob + 1) * 128],
                                 rhs=rrT[:], start=True, stop=True)
            nc.scalar.activation(out=hidT[:, 2 * pb:2 * pb + 2, :], in_=h_ps,
                                 func=AF.Relu)
            for half in range(2):
                ob = 2 * pb + half
                nc.tensor.matmul(out_ps, lhsT=hidT[:, ob, :],
                                 rhs=ow2sb[:, ob, :],
                                 start=(ob == 0), stop=(ob == 9))
        out_sb = cp.tile([128, MED], F32, tag="out_sb")
        nc.vector.tensor_copy(out=out_sb, in_=out_ps)
        dma(out=out_d[:], in_=out_sb)

    if split_waits:
        split_multi_waits(nc)
    return nc


_CACHED_NC = None


def make_in_maps(inputs):
    f = lambda x: np.ascontiguousarray(np.asarray(x, dtype=np.float32))
    bf = lambda x: np.ascontiguousarray(np.asarray(x, dtype=np.float32).astype(BF))
    c_ = np.concatenate

    # out_w1 -> per-core [(f h), m, i] bf16 shards (zero-padded m)
    w1b = f(inputs["out_w1"]).reshape(MED, D, 2, 580)
    w1pad = np.zeros((NC_CORES * MBLK, D, 2, 580), np.float32)
    w1pad[:MED] = w1b
    shards = np.ascontiguousarray(
        w1pad.reshape(NC_CORES, MBLK, D, 2, 580)
        .transpose(0, 2, 3, 1, 4).reshape(NC_CORES, 128, MBLK, 580).astype(F8))

    def tile3(arr2d, n_t, width):
        """[n_t*128, width] row-major -> [128, n_t, width] (k, t, w)."""
        return np.ascontiguousarray(
            arr2d.reshape(n_t, 128, width).transpose(1, 0, 2))

    # lab^T with ones row (bias fold), zero-padded to 2048 rows, tiled
    lab = f(inputs["lab"])
    labT = np.zeros((2048, B), np.float32)
    labT[:LAB] = lab.T
    labT[LAB] = 1.0

    glu = f(inputs["glu"]).reshape(B, T * GLU)
    tf = f(inputs["time_feat"]).reshape(B, T * GLU)
    gluT = np.zeros((512, B), np.float32)
    gluT[:T * GLU] = glu.T
    tfT = np.zeros((512, B), np.float32)
    tfT[:T * GLU] = tf.T

    med0T = np.zeros((146, B), np.float32)
    med0T[:MED] = f(inputs["med"])[:, 0, :].T
    med0T[MED] = 1.0  # > 0.9 threshold -> bias ones row after is_gt

    sll_w1p = np.zeros((2048, D), np.float32)
    sll_w1p[:LAB] = f(inputs["sll_w1"])
    sll_w1p[LAB] = f(inputs["sll_b1"])

    ow2p = np.zeros((1280, MED), np.float32)
    ow2p[:HID] = f(inputs["out_w2"])
    ow2p[HID] = f(inputs["out_b2"])

    head = lambda w: np.ascontiguousarray(
        w.reshape(NH, 16, D).transpose(1, 0, 2))

    rep = {
        "sll_w1p": tile3(sll_w1p, 16, D).astype(BF),
        "sll_w2p": c_([f(inputs["sll_w2"]), f(inputs["sll_b2"]).reshape(1, H)], 0),
        "glu_w16": bf(inputs["glu_w"]),
        "glu_bt": np.ascontiguousarray(
            np.tile(f(inputs["glu_b"]).reshape(1, H), (1, T)).astype(BF)),
        "glu_gate16": bf(inputs["glu_gate"]).reshape(1, H),
        "med_wp": c_([f(inputs["med_w"]), f(inputs["med_b"]).reshape(1, D)],
                     0).astype(BF),
        "med_gate": f(inputs["med_gate"]).reshape(1, D),
        "wq4": head(f(inputs["m1_wq"]).T),
        "wk4": head(f(inputs["m1_wk"]).T),
        "wv4": head(f(inputs["m1_wv"]).T),
        "wo4": head(f(inputs["m1_wo"])),
        "m2wvT": np.ascontiguousarray(f(inputs["m2_wv"]).T),
        "m2wo": f(inputs["m2_wo"]),
        "out_b116": bf(inputs["out_b1"]).reshape(1, HID),
        "ow2p": tile3(ow2p, 10, MED).astype(BF),
    }

    in_maps = []
    for c in range(NC_CORES):
        sl = slice(c * BC, (c + 1) * BC)
        in_maps.append({
            "w1shard": shards[c],
            "labT": tile3(np.ascontiguousarray(labT[:, sl]), 16, 128).astype(BF),
            "gluT": tile3(np.ascontiguousarray(gluT[:, sl]), 4, 128).astype(BF),
            "tfT": tile3(np.ascontiguousarray(tfT[:, sl]), 4, 128).astype(BF),
            "med0T": np.ascontiguousarray(med0T[:, sl]),
            **rep,
        })
    return in_maps


def kernel(**inputs):
    global _CACHED_NC
    if _CACHED_NC is None:
        _CACHED_NC = build_bass()
    nc = _CACHED_NC
    in_maps = make_in_maps(inputs)
    res = run_bass_kernel_spmd(nc, in_maps, core_ids=list(range(NC_CORES)))
    return np.concatenate([res.results[c]["out"] for c in range(NC_CORES)], axis=0)


if __name__ == "__main__":
    import reference
    inp = reference.setup_inputs()
    out = kernel(**{k: np.asarray(v) for k, v in inp.items()})
    print("kernel output", out.shape, out.dtype)
